# revision 24
# baseline (speedup 1.0000x reference)
"""Trainium2 Bass kernel for CustomTransformerEncoderMoELayer (moe_routing).

Sharding: 8 cores = 2 batches x 4 query-quarters. Each core:
  - projects Q^T for its 256 query tokens, K^T/V for its full batch
    (replicated within batch group),
  - z-score softmax attention with matmul-derived row statistics
    (sum S via q.Kbar, sum S^2 via q^T M q, M = K^T K per head),
  - residual + LN1, dense 4-expert MoE (fp8 DoubleRow), residual + LN2.
No cross-core communication; host only shards inputs / concatenates outputs.
"""
import os
import numpy as np
import ml_dtypes

import concourse.bacc as bacc
import concourse.mybir as mybir
import concourse.tile as tile
from concourse.bass_utils import run_bass_kernel_spmd

F32 = mybir.dt.float32
F32R = mybir.dt.float32r
BF16 = mybir.dt.bfloat16
FP8 = mybir.dt.float8e4
PM = mybir.MatmulPerfMode
AF = mybir.ActivationFunctionType
ALU = mybir.AluOpType
AX = mybir.AxisListType

B, T, D, FFD, E, H = 2, 1024, 768, 3072, 4, 12
HD = D // H          # 64
QTOK = 256           # query tokens per core
NCORES = 8
DC = D // 128        # 6 chunks of contraction dim
FT = FFD // 128      # 24 FF tiles
FP = FFD // 256      # 12 fp8 DoubleRow units per expert
NTT = QTOK // 128    # 2 token tiles
KB = T // 128        # 8 key blocks
EPS = 1e-5

_cache = {}
LAST_RESULT = None


def _build(gamma: float, zb: bool):
    """zb: zero-bias/unit-gain fast path (all attn/MoE biases 0, LN gains 1)."""
    nc = bacc.Bacc("TRN2", target_bir_lowering=False, debug=False,
                   num_devices=NCORES)

    # ---- DRAM I/O ----
    d_srcT = nc.dram_tensor("srcT", [D, T], BF16, kind="ExternalInput")
    d_srcq = nc.dram_tensor("srcq", [QTOK, D], F32, kind="ExternalInput")
    d_wqT = nc.dram_tensor("wqT", [D, D], BF16, kind="ExternalInput")
    d_wkT = nc.dram_tensor("wkT", [D, D], BF16, kind="ExternalInput")
    d_wvT = nc.dram_tensor("wvT", [D, D], BF16, kind="ExternalInput")
    d_woT = nc.dram_tensor("woT", [D, D], BF16, kind="ExternalInput")
    d_bqc = nc.dram_tensor("bqc", [128, DC], F32, kind="ExternalInput")
    d_bkc = nc.dram_tensor("bkc", [128, DC], F32, kind="ExternalInput")
    d_ln = nc.dram_tensor("lnrows", [1, 6 * D], BF16, kind="ExternalInput")
    d_wgT = nc.dram_tensor("wgT", [D, E], F32R, kind="ExternalInput")
    d_bgr = nc.dram_tensor("bgr", [1, E], F32R, kind="ExternalInput")
    d_w1 = nc.dram_tensor("w1", [E, 128, FP, 2, 3, 2, 128], FP8,
                          kind="ExternalInput")
    d_w2 = nc.dram_tensor("w2", [E, 128, FP, 2, D], FP8, kind="ExternalInput")
    d_b2r = nc.dram_tensor("b2r", [1, E, D], BF16, kind="ExternalInput")
    d_ident = nc.dram_tensor("ident", [128, 128], F32R, kind="ExternalInput")
    d_ones = nc.dram_tensor("ones_r", [1, 128], F32R, kind="ExternalInput")
    d_out = nc.dram_tensor("out", [QTOK, D], F32, kind="ExternalOutput")

    chunks = [(0, 512), (512, 768)]  # free-dim chunks of D for matmul N<=512

    with tile.TileContext(nc) as tc:
        # ----- persistent pools -----
        cpool = tc.alloc_tile_pool(name="const", bufs=1)
        identr = cpool.tile([128, 128], F32R)
        nc.sync.dma_start(identr[:], d_ident.ap()[:])
        identb = cpool.tile([128, 128], BF16)
        nc.vector.tensor_copy(identb[:], identr[:])
        ones1 = cpool.tile([1, 128], F32R)
        nc.sync.dma_start(ones1[:], d_ones.ap()[:])
        ones2 = cpool.tile([128, 2], BF16)   # col0 = 0, col1 = 1
        nc.vector.memset(ones2[:], 0.0)
        nc.vector.memset(ones2[:, 1:2], 1.0)
        ones1b = cpool.tile([1, 128], BF16)
        nc.vector.memset(ones1b[:], 1.0)
        zerob = cpool.tile([1, 1], BF16)
        nc.vector.memset(zerob[:], 0.0)
        bqc = cpool.tile([128, DC], F32)
        nc.sync.dma_start(bqc[:], d_bqc.ap()[:])
        bkc = cpool.tile([128, DC], F32)
        nc.sync.dma_start(bkc[:], d_bkc.ap()[:])
        bgr = cpool.tile([1, E], F32R)
        nc.sync.dma_start(bgr[:], d_bgr.ap()[:])
        epsc = cpool.tile([128, 1], F32)
        nc.vector.memset(epsc[:], EPS)
        # rows: ln1g ln1b ln2g ln2b bv bo, broadcast across partitions
        lnbc = cpool.tile([128, 6, D], BF16)
        lnp = tc.alloc_tile_pool(name="lnp", bufs=1)
        lnrow = lnp.tile([1, 6 * D], BF16)
        nc.sync.dma_start(lnrow[:], d_ln.ap()[:])
        for i in range(6):
            nc.gpsimd.partition_broadcast(lnbc[:, i, :],
                                          lnrow[0:1, i * D:(i + 1) * D])
        lnp.release()

        bpool = tc.alloc_tile_pool(name="bp", bufs=1)
        attnT = bpool.tile([128, DC, QTOK], BF16)     # attn^T (normalized)
        srcq = bpool.tile([128, NTT, D], F32)         # becomes srcq + bo
        wo = bpool.tile([128, DC, D], BF16)
        wg = bpool.tile([128, DC, E], F32R)

        # MoE expert weights: one DMA per (expert, matrix), ping-pong 2 experts
        mwp0 = tc.alloc_tile_pool(name="mw0", bufs=2)
        west = {}

        def _w_issue(e, eng):
            w1t = mwp0.tile([128, FP, 2, 3, 2, 128], FP8, tag="w1t",
                            name=f"w1e{e}")
            eng.dma_start(w1t[:], d_w1.ap()[e])
            w2t = mwp0.tile([128, FP, 2, D], FP8, tag="w2t", name=f"w2e{e}")
            eng.dma_start(w2t[:], d_w2.ap()[e])
            west[e] = (w1t, w2t)

        perpool = tc.alloc_tile_pool(name="per", bufs=1)
        qT = perpool.tile([128, DC, QTOK], BF16)      # Q^T * scale (+bias)
        kT = perpool.tile([128, DC, T], BF16)         # K^T
        vN = perpool.tile([128, KB, D], BF16)         # V natural [t,d]
        kbar = perpool.tile([128, DC, 1], F32)        # row sums of kT
        kb2 = perpool.tile([128, H, 2], BF16)         # col0 = kbar_h, col1 = 0
        sexp = perpool.tile([128, NTT, H], F32)
        bexp = perpool.tile([128, NTT, H], F32)

        # ================= Phase A: projections =================
        with tc.tile_pool(name="aw", bufs=1) as awp, \
             tc.tile_pool(name="pa1", bufs=3, space="PSUM") as pa1, \
             tc.tile_pool(name="pa2", bufs=2, space="PSUM") as pa2:
            wq = awp.tile([128, DC, D], BF16)
            nc.sync.dma_start(wq[:], d_wqT.ap().rearrange("(c p) d -> p c d", p=128))
            srcT = awp.tile([128, DC, T], BF16)
            nc.sync.dma_start(srcT[:, :, 0:QTOK],
                              d_srcT.ap()[:, 0:QTOK].rearrange("(c p) t -> p c t", p=128))
            nc.sync.dma_start(srcT[:, :, QTOK:T],
                              d_srcT.ap()[:, QTOK:T].rearrange("(c p) t -> p c t", p=128))
            wk = awp.tile([128, DC, D], BF16)
            nc.sync.dma_start(wk[:], d_wkT.ap().rearrange("(c p) d -> p c d", p=128))
            wv = awp.tile([128, DC, D], BF16)
            nc.sync.dma_start(wv[:], d_wvT.ap().rearrange("(c p) d -> p c d", p=128))
            nc.sync.dma_start(srcq[:],
                              d_srcq.ap().rearrange("(tt p) d -> p tt d", p=128))
            nc.sync.dma_start(wo[:], d_woT.ap().rearrange("(c p) d -> p c d", p=128))
            nc.sync.dma_start(wg[:], d_wgT.ap().rearrange("(c p) e -> p c e", p=128))

            # Q^T [D, 256] (pre-scaled by 1/sqrt(hd) on host, incl bias)
            for m in range(DC):
                ps = pa1.tile([128, 512], F32, tag="prj")
                for c in range(DC):
                    nc.tensor.matmul(ps[:, 0:QTOK], wq[:, c, m * 128:(m + 1) * 128],
                                     srcT[:, c, 0:QTOK], start=(c == 0), stop=(c == DC - 1))
                if zb:
                    nc.scalar.copy(qT[:, m, :], ps[:, 0:QTOK])
                else:
                    nc.scalar.activation(qT[:, m, :], ps[:, 0:QTOK], AF.Identity,
                                         bias=bqc[:, m:m + 1])
            # K^T [D, 1024]
            for m in range(DC):
                for kc in range(2):
                    ps = pa1.tile([128, 512], F32, tag="prj")
                    for c in range(DC):
                        nc.tensor.matmul(ps[:], wk[:, c, m * 128:(m + 1) * 128],
                                         srcT[:, c, kc * 512:(kc + 1) * 512],
                                         start=(c == 0), stop=(c == DC - 1))
                    if zb:
                        nc.scalar.copy(kT[:, m, kc * 512:(kc + 1) * 512], ps[:])
                    else:
                        nc.scalar.activation(kT[:, m, kc * 512:(kc + 1) * 512],
                                             ps[:], AF.Identity,
                                             bias=bkc[:, m:m + 1])
            # V natural [T, D]; bias bv added via broadcast row on DVE
            for t8 in range(KB):
                ps = pa2.tile([128, D], F32, tag="vprj")
                for (cs, ce) in chunks:
                    for c in range(DC):
                        nc.tensor.matmul(ps[:, cs:ce], srcT[:, c, t8 * 128:(t8 + 1) * 128],
                                         wv[:, c, cs:ce], start=(c == 0), stop=(c == DC - 1))
                if zb:
                    nc.vector.tensor_copy(vN[:, t8, :], ps[:])
                else:
                    nc.vector.tensor_tensor(vN[:, t8, :], ps[:], lnbc[:, 4, :],
                                            op=ALU.add)
            # row-sums of kT for Sum(S) stats
            nc.vector.reduce_sum(kbar[:], kT[:], axis=AX.X)
            nc.vector.memset(kb2[:], 0.0)
            for h in range(H):
                th, off = (64 * h) // 128, (64 * h) % 128
                nc.vector.tensor_copy(kb2[off:off + 64, h, 0:1],
                                      kbar[off:off + 64, th, :])
            if not zb:  # srcq + bo (used post-Wo)
                for tt in range(NTT):
                    nc.vector.tensor_tensor(srcq[:, tt, :], srcq[:, tt, :],
                                            lnbc[:, 5, :], op=ALU.add)

        # ======== Phase B0: z-score statistics via K moments ========
        # SumS[q]  = q . Kbar         (per head)
        # SumS2[q] = q^T (K^T K) q    (per head)
        spool = tc.alloc_tile_pool(name="sp", bufs=3)
        statp = tc.alloc_tile_pool(name="stat", bufs=1)
        _w_issue(0, nc.gpsimd)
        _w_issue(1, nc.gpsimd)
        with tc.tile_pool(name="kn", bufs=2) as knp, \
             tc.tile_pool(name="pstk", bufs=2, space="PSUM") as pstk, \
             tc.tile_pool(name="pst1", bufs=1, space="PSUM") as pst1, \
             tc.tile_pool(name="stw", bufs=2) as stw:
            st_sb = statp.tile([2, H, QTOK], F32R)     # [stat, head, q]
            for h in range(H):
                th, off = (64 * h) // 128, (64 * h) % 128
                kn = pstk.tile([128, KB, 64], BF16, tag="kn")
                for kb in range(KB):
                    nc.tensor.transpose(kn[:, kb, :],
                                        kT[off:off + 64, th, kb * 128:(kb + 1) * 128],
                                        identb[off:off + 64, off:off + 64])
                knb = knp.tile([128, KB, 64], BF16, tag="knb")
                nc.scalar.copy(knb[:], kn[:])
                mps = pst1.tile([64, 64], F32, tag="mps")
                for kb in range(KB):
                    nc.tensor.matmul(mps[:], knb[:, kb, :], knb[:, kb, :],
                                     start=(kb == 0), stop=(kb == KB - 1))
                msb = stw.tile([128, 64], BF16, tag="msb")
                nc.vector.tensor_copy(msb[off:off + 64, :], mps[:])
                rps = pst1.tile([64, QTOK], F32, tag="rps")
                nc.tensor.matmul(rps[:], msb[off:off + 64, :],
                                 qT[off:off + 64, th, :], start=True, stop=True)
                tpr = stw.tile([128, QTOK], BF16, tag="tpr")
                nc.vector.tensor_tensor(tpr[off:off + 64, :], rps[:],
                                        qT[off:off + 64, th, :], op=ALU.mult)
                pstat = pst1.tile([2, QTOK], F32, tag="sst")
                nc.tensor.matmul(pstat[:], kb2[off:off + 64, h, :],
                                 qT[off:off + 64, th, :], start=True, stop=False)
                nc.tensor.matmul(pstat[:], ones2[off:off + 64, :],
                                 tpr[off:off + 64, :], start=False, stop=True)
                nc.vector.tensor_copy(st_sb[:, h, :], pstat[:])

        # ================= Phase B2: attention =================
        # Software pipelined: S matmuls run one head ahead, PV one head
        # behind, so the tensor queue never waits on the exp/P2/copy chain.
        with tc.tile_pool(name="pba", bufs=1, space="PSUM") as pba, \
             tc.tile_pool(name="pbst", bufs=1, space="PSUM") as pbst, \
             tc.tile_pool(name="pbt", bufs=2, space="PSUM") as pbt, \
             tc.tile_pool(name="pb1", bufs=2, space="PSUM") as pbs, \
             tc.tile_pool(name="st2b", bufs=2) as stp2b:
            Sq = {}
            PTq = {}

            def emit_S(h):
                th, off = (64 * h) // 128, (64 * h) % 128
                for qt in range(NTT):
                    S = pbs.tile([128, T], F32, tag="S")
                    for kc in range(2):
                        nc.tensor.matmul(
                            S[:, kc * 512:(kc + 1) * 512],
                            qT[off:off + 64, th, qt * 128:(qt + 1) * 128],
                            kT[off:off + 64, th, kc * 512:(kc + 1) * 512],
                            start=True, stop=True)
                    Sq[(h, qt)] = S

            def emit_soft(h):
                # exp + normalize + transpose + copy for head h
                den = stp2b.tile([128, NTT], F32, tag="den")
                rden = stp2b.tile([128, NTT], F32, tag="rden")
                PTsb = spool.tile([128, NTT, KB, 128], BF16, tag="PTsb")
                for qt in range(NTT):
                    S = Sq.pop((h, qt))
                    P = spool.tile([128, T], BF16, tag="P")
                    nc.scalar.activation(P[:], S[:], AF.Exp,
                                         bias=bexp[:, qt, h:h + 1],
                                         scale=sexp[:, qt, h:h + 1],
                                         accum_out=den[:, qt:qt + 1])
                    nc.vector.reciprocal(rden[:, qt:qt + 1], den[:, qt:qt + 1])
                    P2 = spool.tile([128, T], BF16, tag="P2")
                    nc.vector.tensor_scalar(P2[:], P[:], rden[:, qt:qt + 1],
                                            None, op0=ALU.mult)
                    PT = pbt.tile([128, KB, 128], BF16, tag="PT")
                    for kb in range(KB):
                        nc.tensor.transpose(PT[:, kb, :],
                                            P2[:, kb * 128:(kb + 1) * 128],
                                            identb[:])
                    nc.vector.tensor_copy(PTsb[:, qt], PT[:])
                PTq[h] = PTsb

            def emit_pv(h):
                th, off = (64 * h) // 128, (64 * h) % 128
                PTsb = PTq.pop(h)
                aps = pba.tile([64, QTOK], F32, tag="aps")
                for kb in range(KB):
                    nc.tensor.matmul(aps[:], vN[:, kb, h * 64:(h + 1) * 64],
                                     PTsb[:, :, kb, :], start=(kb == 0),
                                     stop=(kb == KB - 1))
                nc.vector.tensor_copy(attnT[off:off + 64, th, :], aps[:])

            emit_S(0)
            # stats finalize: transpose to [q, (h, stat)], exp scale/bias
            stT = statp.tile([128, NTT, H, 2], F32)
            for qt in range(NTT):
                stps = pbst.tile([128, H, 2], F32R, tag="stps")
                for h in range(H):
                    nc.tensor.transpose(stps[:, h, :],
                                        st_sb[:, h, qt * 128:(qt + 1) * 128],
                                        identr[0:2, 0:2])
                nc.vector.tensor_copy(stT[:, qt], stps[:])
            for qt in range(NTT):
                sums = stT[:, qt, :, 0]
                sqs = stT[:, qt, :, 1]
                mean = stp2b.tile([128, H], F32, tag="mean")
                tm1 = stp2b.tile([128, H], F32, tag="tm1")
                sig = stp2b.tile([128, H], F32, tag="sig")
                nc.vector.tensor_scalar_mul(mean[:], sums, 1.0 / T)
                nc.vector.tensor_tensor(tm1[:], sums, mean[:], op=ALU.mult)
                nc.vector.tensor_sub(tm1[:], sqs, tm1[:])
                nc.scalar.activation(sig[:], tm1[:], AF.Sqrt, scale=1.0 / (T - 1))
                nc.vector.tensor_scalar_add(sig[:], sig[:], EPS)
                nc.vector.reciprocal(sexp[:, qt, :], sig[:])
                if gamma != 1.0:
                    nc.vector.tensor_scalar_mul(sexp[:, qt, :], sexp[:, qt, :],
                                                float(gamma))
                nc.vector.scalar_tensor_tensor(bexp[:, qt, :], mean[:], -1.0,
                                               sexp[:, qt, :],
                                               op0=ALU.mult, op1=ALU.mult)
            for h in range(H):
                if h + 1 < H:
                    emit_S(h + 1)
                emit_soft(h)
                if h >= 1:
                    emit_pv(h - 1)
            emit_pv(H - 1)
        statp.release()
        spool.release()

        # ============ Phase C: Wo + LN1 + x^T + gate ============
        wpool = tc.alloc_tile_pool(name="cw", bufs=1)
        with tc.tile_pool(name="st2", bufs=2) as stp2:
          with tc.tile_pool(name="pc1", bufs=2, space="PSUM") as pc1:
              if not zb:
                  b2r = wpool.tile([1, E, D], BF16)
                  nc.sync.dma_start(b2r[:], d_b2r.ap()[:])
              # e2/e3 expert weights (gpsimd queue blocks on WAR until
              # e0/e1 are consumed mid phase D -- gpsimd is idle then)
              _w_issue(2, nc.gpsimd)
              _w_issue(3, nc.gpsimd)
              x_sb = wpool.tile([128, NTT, D], F32R)      # post-LN1
              xT32 = wpool.tile([128, DC, NTT, 128], F32R)  # x^T f32 (gate)
              xT8 = wpool.tile([128, DC, NTT, 128], FP8)  # x^T * 16 (fp8)
              comb = wpool.tile([128, NTT, E], F32)       # top-2 combine weights
              ffs = wpool.tile([128, NTT, D], F32)
              out_sb = wpool.tile([128, NTT, D], F32)

              def layer_norm(dst_ap, pre_ap, gb_idx):
                  s1 = stp2.tile([128, 1], F32, tag="s1")
                  q1 = stp2.tile([128, 1], F32, tag="q1")
                  mn = stp2.tile([128, 1], F32, tag="mn")
                  vv = stp2.tile([128, 1], F32, tag="vv")
                  rs = stp2.tile([128, 1], F32, tag="rs")
                  bb = stp2.tile([128, 1], F32, tag="bb")
                  xn = stp2.tile([128, D], F32, tag="xn")
                  sq2 = stp2.tile([128, D], F32, tag="xn")
                  nc.vector.reduce_sum(s1[:], pre_ap, axis=AX.X)
                  nc.scalar.activation(sq2[:], pre_ap, AF.Square, accum_out=q1[:])
                  nc.vector.tensor_scalar_mul(mn[:], s1[:], 1.0 / D)
                  nc.vector.tensor_tensor(vv[:], mn[:], mn[:], op=ALU.mult)
                  nc.vector.scalar_tensor_tensor(vv[:], q1[:], 1.0 / D, vv[:],
                                                 op0=ALU.mult, op1=ALU.subtract)
                  sr = stp2.tile([128, 1], F32, tag="sr")
                  nc.scalar.activation(sr[:], vv[:], AF.Sqrt, bias=epsc[:])
                  nc.vector.reciprocal(rs[:], sr[:])
                  nc.vector.scalar_tensor_tensor(bb[:], mn[:], -1.0, rs[:],
                                                 op0=ALU.mult, op1=ALU.mult)
                  if zb:
                      nc.scalar.activation(dst_ap, pre_ap, AF.Identity,
                                           bias=bb[:], scale=rs[:])
                  else:
                      nc.scalar.activation(xn[:], pre_ap, AF.Identity,
                                           bias=bb[:], scale=rs[:])
                      nc.vector.tensor_tensor(xn[:], xn[:],
                                              lnbc[:, 2 * gb_idx, :], op=ALU.mult)
                      nc.vector.tensor_tensor(dst_ap, xn[:],
                                              lnbc[:, 2 * gb_idx + 1, :],
                                              op=ALU.add)

              for tt in range(NTT):
                  ps = pc1.tile([128, D], F32, tag="wo")
                  for (cs, ce) in chunks:
                      for c in range(DC):
                          nc.tensor.matmul(ps[:, cs:ce],
                                           attnT[:, c, tt * 128:(tt + 1) * 128],
                                           wo[:, c, cs:ce], start=(c == 0),
                                           stop=(c == DC - 1))
                  pre = stp2.tile([128, D], F32, tag="pre")
                  nc.vector.tensor_tensor(pre[:], ps[:], srcq[:, tt, :], op=ALU.add)
                  layer_norm(x_sb[:, tt, :], pre[:], 0)

              # x^T
              for c in range(DC):
                  xtp = pc1.tile([128, NTT, 128], F32R, tag="xtp")
                  for tt in range(NTT):
                      nc.tensor.transpose(xtp[:, tt, :],
                                          x_sb[:, tt, c * 128:(c + 1) * 128], identr[:])
                  nc.vector.tensor_copy(xT32[:, c], xtp[:])
                  nc.scalar.mul(xT8[:, c], xtp[:], 16.0)

              # gate + top-2 combine
              for tt in range(NTT):
                  gp = pc1.tile([128, E], F32, tag="gate")
                  for c in range(DC):
                      nc.tensor.matmul(gp[:], xT32[:, c, tt, :], wg[:, c, :],
                                       start=(c == 0),
                                       stop=(zb and c == DC - 1))
                  if not zb:
                      nc.tensor.matmul(gp[:], ones1[0:1, :], bgr[0:1, :],
                                       start=False, stop=True)
                  mx = stp2.tile([128, 1], F32, tag="mx")
                  se = stp2.tile([128, 1], F32, tag="se")
                  eg = stp2.tile([128, E], F32, tag="eg")
                  pr = stp2.tile([128, E], F32, tag="pr")
                  m2 = stp2.tile([128, 1], F32, tag="m2")
                  kp = stp2.tile([128, E], F32, tag="kp")
                  nc.vector.reduce_max(mx[:], gp[:], axis=AX.X)
                  nc.vector.tensor_scalar_mul(mx[:], mx[:], -1.0)
                  nc.scalar.activation(eg[:], gp[:], AF.Exp, bias=mx[:], accum_out=se[:])
                  nc.vector.reciprocal(se[:], se[:])
                  nc.vector.tensor_scalar_mul(pr[:], eg[:], se[:])
                  nc.vector.reduce_max(mx[:], pr[:], axis=AX.X)
                  nc.vector.tensor_scalar(kp[:], pr[:], mx[:], None, op0=ALU.is_ge)
                  nc.vector.scalar_tensor_tensor(eg[:], kp[:], -1e9, pr[:],
                                                 op0=ALU.mult, op1=ALU.add)
                  nc.vector.reduce_max(m2[:], eg[:], axis=AX.X)
                  nc.vector.tensor_scalar(kp[:], pr[:], m2[:], None, op0=ALU.is_ge)
                  nc.vector.tensor_tensor(comb[:, tt, :], pr[:], kp[:], op=ALU.mult)
                  nc.vector.tensor_scalar(comb[:, tt, :], comb[:, tt, :],
                                          2.0 ** -14, None, op0=ALU.mult)

          # ============ Phase D: MoE experts ============
          with tc.tile_pool(name="mh", bufs=3) as mhp, \
               tc.tile_pool(name="pd1", bufs=3, space="PSUM") as pd1, \
               tc.tile_pool(name="pd2", bufs=1, space="PSUM") as pd2:
              for e in range(E):
                  w1t, w2t = west[e]
                  yps = [pd2.tile([128, D], F32, tag=f"y{tt}", name=f"y{tt}") for tt in range(NTT)]
                  for fp in range(FP):
                      hp = pd1.tile([128, 2, QTOK], F32, tag="hps")
                      for t in range(2):
                          for cp in range(3):
                              nc.tensor.matmul(hp[:, t, :], w1t[:, fp, t, cp],
                                               xT8[:, 2 * cp:2 * cp + 2]
                                               .rearrange("p c t f -> p c (t f)"),
                                               start=(cp == 0), stop=(cp == 2),
                                               perf_mode=PM.DoubleRow)
                      hsb = mhp.tile([128, 2, QTOK], FP8, tag="hsb")
                      if fp % 2 == 0:
                          nc.scalar.activation(hsb[:], hp[:], AF.Relu,
                                               scale=2.0 ** -6)
                      else:
                          nc.vector.tensor_scalar(hsb[:], hp[:], 2.0 ** -6,
                                                  0.0, op0=ALU.mult,
                                                  op1=ALU.max)
                      for tt in range(NTT):
                          for (cs, ce) in chunks:
                              nc.tensor.matmul(yps[tt][:, cs:ce],
                                               hsb[:, :, tt * 128:(tt + 1) * 128],
                                               w2t[:, fp, :, cs:ce],
                                               start=(fp == 0),
                                               stop=(zb and fp == FP - 1),
                                               perf_mode=PM.DoubleRow)
                  for tt in range(NTT):
                      if not zb:
                          for (cs, ce) in chunks:
                              nc.tensor.matmul(yps[tt][:, cs:ce], ones1b[0:1, :],
                                               b2r[0:1, e, cs:ce],
                                               start=False, stop=True)
                      if e == 0:
                          nc.scalar.mul(ffs[:, tt, :], yps[tt][:],
                                        comb[:, tt, e:e + 1])
                      else:
                          nc.vector.scalar_tensor_tensor(
                              ffs[:, tt, :], yps[tt][:], comb[:, tt, e:e + 1],
                              ffs[:, tt, :], op0=ALU.mult, op1=ALU.add)

              for tt in range(NTT):
                  pre2 = stp2.tile([128, D], F32, tag="pre")
                  nc.vector.tensor_tensor(pre2[:], x_sb[:, tt, :], ffs[:, tt, :],
                                          op=ALU.add)
                  layer_norm(out_sb[:, tt, :], pre2[:], 1)
              for tt in range(NTT):
                  nc.sync.dma_start(
                      d_out.ap()[tt * 128:(tt + 1) * 128].rearrange(
                          "(o p) d -> p o d", p=128), out_sb[:, tt:tt + 1, :])
        wpool.release()
        perpool.release()
        mwp0.release()
        bpool.release()
        cpool.release()

    nc.compile()
    return nc


def _prep(inputs):
    f = lambda a: np.ascontiguousarray(np.asarray(a, dtype=np.float32))
    bf = lambda a: np.ascontiguousarray(a).astype(ml_dtypes.bfloat16)
    src = f(inputs["src"])
    scale = (D // H) ** -0.5
    lnrows = np.concatenate([
        f(inputs["ln1_g"]), f(inputs["ln1_b"]),
        f(inputs["ln2_g"]), f(inputs["ln2_b"]),
        f(inputs["bv"]), f(inputs["bo"])]).reshape(1, 6 * D)
    common = {
        "wqT": bf(f(inputs["Wq"]).T * scale),
        "wkT": bf(f(inputs["Wk"]).T),
        "wvT": bf(f(inputs["Wv"]).T),
        "woT": bf(f(inputs["Wo"]).T),
        "bqc": (f(inputs["bq"]) * scale).reshape(DC, 128).T.copy(),
        "bkc": f(inputs["bk"]).reshape(DC, 128).T.copy(),
        "lnrows": bf(lnrows),
        "wgT": np.ascontiguousarray(f(inputs["Wg"]).T),
        "bgr": f(inputs["bg"]).reshape(1, E),
        "w1": np.ascontiguousarray(
            (f(inputs["W1"]) * 256.0).reshape(E, 3, 2, 128, FP, 2, 128)
            .transpose(0, 3, 4, 5, 1, 2, 6)).astype(np.dtype("float8_e4m3")),
        "w2": np.ascontiguousarray(
            (f(inputs["W2"]) * 256.0).reshape(E, FP, 2, 128, D)
            .transpose(0, 3, 1, 2, 4)).astype(np.dtype("float8_e4m3")),
        "b2r": bf(f(inputs["b2"]).reshape(1, E, D) * (2.0 ** 14)),
        "ident": np.eye(128, dtype=np.float32),
        "ones_r": np.ones((1, 128), dtype=np.float32),
    }
    assert not np.any(f(inputs["b1"])), "fp8 MoE path requires zero b1"
    in_maps = []
    for c in range(NCORES):
        b, qq = c // 4, c % 4
        m = dict(common)
        # rotate key/value token axis so this core's quarter sits at cols 0:256
        m["srcT"] = bf(np.roll(src[b].T, -qq * QTOK, axis=1))
        m["srcq"] = np.ascontiguousarray(src[b, qq * QTOK:(qq + 1) * QTOK])
        in_maps.append(m)
    return in_maps


def kernel(**inputs):
    global LAST_RESULT
    gamma = float(np.asarray(inputs["gamma"]))
    zb = (not any(np.any(np.asarray(inputs[k])) for k in
                  ("bq", "bk", "bv", "bo", "bg", "b2", "ln1_b", "ln2_b"))
          and all(np.all(np.asarray(inputs[k]) == 1.0) for k in ("ln1_g", "ln2_g")))
    key = (round(gamma, 9), zb)
    if key not in _cache:
        _cache[key] = _build(gamma, zb)
    nc = _cache[key]
    in_maps = _prep(inputs)
    trace = bool(os.environ.get("KERNEL_TRACE"))
    try:
        res = run_bass_kernel_spmd(nc, in_maps, list(range(NCORES)), trace=trace)
    except ModuleNotFoundError:
        res = run_bass_kernel_spmd(nc, in_maps, list(range(NCORES)), trace=False)
    LAST_RESULT = res
    out = np.empty((B, T, D), dtype=np.float32)
    for c in range(NCORES):
        b, qq = c // 4, c % 4
        out[b, qq * QTOK:(qq + 1) * QTOK] = res.results[c]["out"]
    return out


# revision 28
# speedup vs baseline: 1.0148x; 1.0148x over previous
"""Trainium2 Bass kernel for CustomTransformerEncoderMoELayer (moe_routing).

Sharding: 8 cores = 2 batches x 4 query-quarters. Each core:
  - projects Q^T for its 256 query tokens, K^T/V for its full batch
    (replicated within batch group),
  - z-score softmax attention with matmul-derived row statistics
    (sum S via q.Kbar, sum S^2 via q^T M q, M = K^T K per head),
  - residual + LN1, dense 4-expert MoE (fp8 DoubleRow), residual + LN2.
No cross-core communication; host only shards inputs / concatenates outputs.
"""
import os
import numpy as np
import ml_dtypes

import concourse.bacc as bacc
import concourse.mybir as mybir
import concourse.tile as tile
from concourse.bass_utils import run_bass_kernel_spmd

F32 = mybir.dt.float32
F32R = mybir.dt.float32r
BF16 = mybir.dt.bfloat16
FP8 = mybir.dt.float8e4
PM = mybir.MatmulPerfMode
AF = mybir.ActivationFunctionType
ALU = mybir.AluOpType
AX = mybir.AxisListType

B, T, D, FFD, E, H = 2, 1024, 768, 3072, 4, 12
HD = D // H          # 64
QTOK = 256           # query tokens per core
NCORES = 8
DC = D // 128        # 6 chunks of contraction dim
FT = FFD // 128      # 24 FF tiles
FP = FFD // 256      # 12 fp8 DoubleRow units per expert
NTT = QTOK // 128    # 2 token tiles
KB = T // 128        # 8 key blocks
EPS = 1e-5

_cache = {}
LAST_RESULT = None


def _build(gamma: float, zb: bool):
    """zb: zero-bias/unit-gain fast path (all attn/MoE biases 0, LN gains 1)."""
    nc = bacc.Bacc("TRN2", target_bir_lowering=False, debug=False,
                   num_devices=NCORES)

    # ---- DRAM I/O ----
    d_srcT = nc.dram_tensor("srcT", [D, T], BF16, kind="ExternalInput")
    d_srcq = nc.dram_tensor("srcq", [QTOK, D], F32, kind="ExternalInput")
    d_wqT = nc.dram_tensor("wqT", [D, D], BF16, kind="ExternalInput")
    d_wkT = nc.dram_tensor("wkT", [D, D], BF16, kind="ExternalInput")
    d_wvT = nc.dram_tensor("wvT", [D, D], BF16, kind="ExternalInput")
    d_woT = nc.dram_tensor("woT", [D, D], BF16, kind="ExternalInput")
    d_bqc = nc.dram_tensor("bqc", [128, DC], F32, kind="ExternalInput")
    d_bkc = nc.dram_tensor("bkc", [128, DC], F32, kind="ExternalInput")
    d_ln = nc.dram_tensor("lnrows", [1, 6 * D], BF16, kind="ExternalInput")
    d_wgT = nc.dram_tensor("wgT", [D, E], F32R, kind="ExternalInput")
    d_bgr = nc.dram_tensor("bgr", [1, E], F32R, kind="ExternalInput")
    d_w1 = nc.dram_tensor("w1", [E, 128, FP, 2, 3, 2, 128], FP8,
                          kind="ExternalInput")
    d_w2 = nc.dram_tensor("w2", [E, 128, FP, 2, D], FP8, kind="ExternalInput")
    d_b2r = nc.dram_tensor("b2r", [1, E, D], BF16, kind="ExternalInput")
    d_ident = nc.dram_tensor("ident", [128, 128], F32R, kind="ExternalInput")
    d_ones = nc.dram_tensor("ones_r", [1, 128], F32R, kind="ExternalInput")
    d_out = nc.dram_tensor("out", [QTOK, D], F32, kind="ExternalOutput")

    chunks = [(0, 512), (512, 768)]  # free-dim chunks of D for matmul N<=512

    with tile.TileContext(nc) as tc:
        # ----- persistent pools -----
        cpool = tc.alloc_tile_pool(name="const", bufs=1)
        identr = cpool.tile([128, 128], F32R)
        nc.sync.dma_start(identr[:], d_ident.ap()[:])
        identb = cpool.tile([128, 128], BF16)
        nc.vector.tensor_copy(identb[:], identr[:])
        ones1 = cpool.tile([1, 128], F32R)
        nc.sync.dma_start(ones1[:], d_ones.ap()[:])
        ones2 = cpool.tile([128, 2], BF16)   # col0 = 0, col1 = 1
        nc.vector.memset(ones2[:], 0.0)
        nc.vector.memset(ones2[:, 1:2], 1.0)
        ones1b = cpool.tile([1, 128], BF16)
        nc.vector.memset(ones1b[:], 1.0)
        zerob = cpool.tile([1, 1], BF16)
        nc.vector.memset(zerob[:], 0.0)
        bqc = cpool.tile([128, DC], F32)
        nc.sync.dma_start(bqc[:], d_bqc.ap()[:])
        bkc = cpool.tile([128, DC], F32)
        nc.sync.dma_start(bkc[:], d_bkc.ap()[:])
        bgr = cpool.tile([1, E], F32R)
        nc.sync.dma_start(bgr[:], d_bgr.ap()[:])
        epsc = cpool.tile([128, 1], F32)
        nc.vector.memset(epsc[:], EPS)
        # rows: ln1g ln1b ln2g ln2b bv bo, broadcast across partitions
        lnbc = cpool.tile([128, 6, D], BF16)
        lnp = tc.alloc_tile_pool(name="lnp", bufs=1)
        lnrow = lnp.tile([1, 6 * D], BF16)
        nc.sync.dma_start(lnrow[:], d_ln.ap()[:])
        for i in range(6):
            nc.gpsimd.partition_broadcast(lnbc[:, i, :],
                                          lnrow[0:1, i * D:(i + 1) * D])
        lnp.release()

        bpool = tc.alloc_tile_pool(name="bp", bufs=1)
        attnT = bpool.tile([128, DC, QTOK], BF16)     # attn^T (normalized)
        srcq = bpool.tile([128, NTT, D], F32)         # becomes srcq + bo
        wo = bpool.tile([128, DC, D], BF16)
        wg = bpool.tile([128, DC, E], F32R)

        # MoE expert weights: one DMA per (expert, matrix), ping-pong 2 experts
        mwp0 = tc.alloc_tile_pool(name="mw0", bufs=2)
        west = {}

        def _w_issue(e, eng):
            w1t = mwp0.tile([128, FP, 2, 3, 2, 128], FP8, tag="w1t",
                            name=f"w1e{e}")
            eng.dma_start(w1t[:], d_w1.ap()[e])
            w2t = mwp0.tile([128, FP, 2, D], FP8, tag="w2t", name=f"w2e{e}")
            eng.dma_start(w2t[:], d_w2.ap()[e])
            west[e] = (w1t, w2t)

        perpool = tc.alloc_tile_pool(name="per", bufs=1)
        qT = perpool.tile([128, DC, QTOK], BF16)      # Q^T * scale (+bias)
        kT = perpool.tile([128, DC, T], BF16)         # K^T
        vN = perpool.tile([128, KB, D], BF16)         # V natural [t,d]
        kbar = perpool.tile([128, DC, 1], F32)        # row sums of kT
        kb2 = perpool.tile([128, H, 2], BF16)         # col0 = kbar_h, col1 = 0
        sexp = perpool.tile([128, NTT, H], F32)
        bexp = perpool.tile([128, NTT, H], F32)

        # ================= Phase A: projections =================
        with tc.tile_pool(name="aw", bufs=1) as awp, \
             tc.tile_pool(name="pa1", bufs=3, space="PSUM") as pa1, \
             tc.tile_pool(name="pa2", bufs=2, space="PSUM") as pa2:
            wq = awp.tile([128, DC, D], BF16)
            nc.sync.dma_start(wq[:], d_wqT.ap().rearrange("(c p) d -> p c d", p=128))
            srcT = awp.tile([128, DC, T], BF16)
            nc.sync.dma_start(srcT[:, :, 0:QTOK],
                              d_srcT.ap()[:, 0:QTOK].rearrange("(c p) t -> p c t", p=128))
            nc.sync.dma_start(srcT[:, :, QTOK:T],
                              d_srcT.ap()[:, QTOK:T].rearrange("(c p) t -> p c t", p=128))
            wk = awp.tile([128, DC, D], BF16)
            nc.sync.dma_start(wk[:], d_wkT.ap().rearrange("(c p) d -> p c d", p=128))
            wv = awp.tile([128, DC, D], BF16)
            nc.sync.dma_start(wv[:], d_wvT.ap().rearrange("(c p) d -> p c d", p=128))
            nc.sync.dma_start(srcq[:],
                              d_srcq.ap().rearrange("(tt p) d -> p tt d", p=128))
            nc.sync.dma_start(wo[:], d_woT.ap().rearrange("(c p) d -> p c d", p=128))
            nc.sync.dma_start(wg[:], d_wgT.ap().rearrange("(c p) e -> p c e", p=128))

            # Q^T [D, 256] (pre-scaled by 1/sqrt(hd) on host, incl bias)
            for m in range(DC):
                ps = pa1.tile([128, 512], F32, tag="prj")
                for c in range(DC):
                    nc.tensor.matmul(ps[:, 0:QTOK], wq[:, c, m * 128:(m + 1) * 128],
                                     srcT[:, c, 0:QTOK], start=(c == 0), stop=(c == DC - 1))
                if zb:
                    nc.scalar.copy(qT[:, m, :], ps[:, 0:QTOK])
                else:
                    nc.scalar.activation(qT[:, m, :], ps[:, 0:QTOK], AF.Identity,
                                         bias=bqc[:, m:m + 1])
            # K^T [D, 1024]
            for m in range(DC):
                for kc in range(2):
                    ps = pa1.tile([128, 512], F32, tag="prj")
                    for c in range(DC):
                        nc.tensor.matmul(ps[:], wk[:, c, m * 128:(m + 1) * 128],
                                         srcT[:, c, kc * 512:(kc + 1) * 512],
                                         start=(c == 0), stop=(c == DC - 1))
                    if zb:
                        nc.scalar.copy(kT[:, m, kc * 512:(kc + 1) * 512], ps[:])
                    else:
                        nc.scalar.activation(kT[:, m, kc * 512:(kc + 1) * 512],
                                             ps[:], AF.Identity,
                                             bias=bkc[:, m:m + 1])
            # V natural [T, D]; bias bv added via broadcast row on DVE
            for t8 in range(KB):
                ps = pa2.tile([128, D], F32, tag="vprj")
                for (cs, ce) in chunks:
                    for c in range(DC):
                        nc.tensor.matmul(ps[:, cs:ce], srcT[:, c, t8 * 128:(t8 + 1) * 128],
                                         wv[:, c, cs:ce], start=(c == 0), stop=(c == DC - 1))
                if zb:
                    nc.vector.tensor_copy(vN[:, t8, :], ps[:])
                else:
                    nc.vector.tensor_tensor(vN[:, t8, :], ps[:], lnbc[:, 4, :],
                                            op=ALU.add)
            # row-sums of kT for Sum(S) stats
            nc.vector.reduce_sum(kbar[:], kT[:], axis=AX.X)
            nc.vector.memset(kb2[:], 0.0)
            for h in range(H):
                th, off = (64 * h) // 128, (64 * h) % 128
                nc.vector.tensor_copy(kb2[off:off + 64, h, 0:1],
                                      kbar[off:off + 64, th, :])
            if not zb:  # srcq + bo (used post-Wo)
                for tt in range(NTT):
                    nc.vector.tensor_tensor(srcq[:, tt, :], srcq[:, tt, :],
                                            lnbc[:, 5, :], op=ALU.add)

        # ======== Phase B0: z-score statistics via K moments ========
        # SumS[q]  = q . Kbar         (per head)
        # SumS2[q] = q^T (K^T K) q    (per head)
        spool = tc.alloc_tile_pool(name="sp", bufs=3)
        statp = tc.alloc_tile_pool(name="stat", bufs=1)
        _w_issue(0, nc.gpsimd)
        _w_issue(1, nc.gpsimd)
        with tc.tile_pool(name="kn", bufs=2) as knp, \
             tc.tile_pool(name="pstk", bufs=2, space="PSUM") as pstk, \
             tc.tile_pool(name="pst1", bufs=1, space="PSUM") as pst1, \
             tc.tile_pool(name="stw", bufs=2) as stw:
            st_sb = statp.tile([2, H, QTOK], F32R)     # [stat, head, q]
            for h in range(H):
                th, off = (64 * h) // 128, (64 * h) % 128
                kn = pstk.tile([128, KB, 64], BF16, tag="kn")
                for kb in range(KB):
                    nc.tensor.transpose(kn[:, kb, :],
                                        kT[off:off + 64, th, kb * 128:(kb + 1) * 128],
                                        identb[off:off + 64, off:off + 64])
                knb = knp.tile([128, KB, 64], BF16, tag="knb")
                nc.scalar.copy(knb[:], kn[:])
                mps = pst1.tile([64, 64], F32, tag="mps")
                for kb in range(KB):
                    nc.tensor.matmul(mps[:], knb[:, kb, :], knb[:, kb, :],
                                     start=(kb == 0), stop=(kb == KB - 1))
                msb = stw.tile([128, 64], BF16, tag="msb")
                nc.vector.tensor_copy(msb[off:off + 64, :], mps[:])
                rps = pst1.tile([64, QTOK], F32, tag="rps")
                nc.tensor.matmul(rps[:], msb[off:off + 64, :],
                                 qT[off:off + 64, th, :], start=True, stop=True)
                tpr = stw.tile([128, QTOK], BF16, tag="tpr")
                nc.vector.tensor_tensor(tpr[off:off + 64, :], rps[:],
                                        qT[off:off + 64, th, :], op=ALU.mult)
                pstat = pst1.tile([2, QTOK], F32, tag="sst")
                nc.tensor.matmul(pstat[:], kb2[off:off + 64, h, :],
                                 qT[off:off + 64, th, :], start=True, stop=False)
                nc.tensor.matmul(pstat[:], ones2[off:off + 64, :],
                                 tpr[off:off + 64, :], start=False, stop=True)
                nc.vector.tensor_copy(st_sb[:, h, :], pstat[:])

        # ================= Phase B2: attention =================
        # Software pipelined: S matmuls run one head ahead, PV one head
        # behind, so the tensor queue never waits on the exp/P2/copy chain.
        with tc.tile_pool(name="pba", bufs=1, space="PSUM") as pba, \
             tc.tile_pool(name="pbst", bufs=1, space="PSUM") as pbst, \
             tc.tile_pool(name="pbt", bufs=2, space="PSUM") as pbt, \
             tc.tile_pool(name="pb1", bufs=2, space="PSUM") as pbs, \
             tc.tile_pool(name="st2b", bufs=2) as stp2b:
            Sq = {}
            PTq = {}

            def emit_S(h):
                th, off = (64 * h) // 128, (64 * h) % 128
                for qt in range(NTT):
                    S = pbs.tile([128, T], F32, tag="S")
                    for kc in range(2):
                        nc.tensor.matmul(
                            S[:, kc * 512:(kc + 1) * 512],
                            qT[off:off + 64, th, qt * 128:(qt + 1) * 128],
                            kT[off:off + 64, th, kc * 512:(kc + 1) * 512],
                            start=True, stop=True)
                    Sq[(h, qt)] = S

            def emit_soft(h):
                # exp + normalize + transpose + copy for head h
                den = stp2b.tile([128, NTT], F32, tag="den")
                rden = stp2b.tile([128, NTT], F32, tag="rden")
                PTsb = spool.tile([128, NTT, KB, 128], BF16, tag="PTsb")
                for qt in range(NTT):
                    S = Sq.pop((h, qt))
                    P = spool.tile([128, T], BF16, tag="P")
                    nc.scalar.activation(P[:], S[:], AF.Exp,
                                         bias=bexp[:, qt, h:h + 1],
                                         scale=sexp[:, qt, h:h + 1],
                                         accum_out=den[:, qt:qt + 1])
                    nc.vector.reciprocal(rden[:, qt:qt + 1], den[:, qt:qt + 1])
                    P2 = spool.tile([128, T], BF16, tag="P2")
                    nc.vector.tensor_scalar(P2[:], P[:], rden[:, qt:qt + 1],
                                            None, op0=ALU.mult)
                    PT = pbt.tile([128, KB, 128], BF16, tag="PT")
                    for kb in range(KB):
                        nc.tensor.transpose(PT[:, kb, :],
                                            P2[:, kb * 128:(kb + 1) * 128],
                                            identb[:])
                    nc.vector.tensor_copy(PTsb[:, qt], PT[:])
                PTq[h] = PTsb

            def emit_pv(h):
                th, off = (64 * h) // 128, (64 * h) % 128
                PTsb = PTq.pop(h)
                aps = pba.tile([64, QTOK], F32, tag="aps")
                for kb in range(KB):
                    nc.tensor.matmul(aps[:], vN[:, kb, h * 64:(h + 1) * 64],
                                     PTsb[:, :, kb, :], start=(kb == 0),
                                     stop=(kb == KB - 1))
                nc.vector.tensor_copy(attnT[off:off + 64, th, :], aps[:])

            emit_S(0)
            # stats finalize: transpose to [q, (h, stat)], exp scale/bias
            stT = statp.tile([128, NTT, H, 2], F32)
            for qt in range(NTT):
                stps = pbst.tile([128, H, 2], F32R, tag="stps")
                for h in range(H):
                    nc.tensor.transpose(stps[:, h, :],
                                        st_sb[:, h, qt * 128:(qt + 1) * 128],
                                        identr[0:2, 0:2])
                nc.vector.tensor_copy(stT[:, qt], stps[:])
            for qt in range(NTT):
                sums = stT[:, qt, :, 0]
                sqs = stT[:, qt, :, 1]
                mean = stp2b.tile([128, H], F32, tag="mean")
                tm1 = stp2b.tile([128, H], F32, tag="tm1")
                sig = stp2b.tile([128, H], F32, tag="sig")
                nc.vector.tensor_scalar_mul(mean[:], sums, 1.0 / T)
                nc.vector.tensor_tensor(tm1[:], sums, mean[:], op=ALU.mult)
                nc.vector.tensor_sub(tm1[:], sqs, tm1[:])
                nc.scalar.activation(sig[:], tm1[:], AF.Sqrt, scale=1.0 / (T - 1))
                nc.vector.tensor_scalar_add(sig[:], sig[:], EPS)
                nc.vector.reciprocal(sexp[:, qt, :], sig[:])
                if gamma != 1.0:
                    nc.vector.tensor_scalar_mul(sexp[:, qt, :], sexp[:, qt, :],
                                                float(gamma))
                nc.vector.scalar_tensor_tensor(bexp[:, qt, :], mean[:], -1.0,
                                               sexp[:, qt, :],
                                               op0=ALU.mult, op1=ALU.mult)
            for h in range(H):
                if h + 1 < H:
                    emit_S(h + 1)
                emit_soft(h)
                if h >= 1:
                    emit_pv(h - 1)
            emit_pv(H - 1)
        statp.release()
        spool.release()

        # ============ Phase C: Wo + LN1 + x^T + gate ============
        wpool = tc.alloc_tile_pool(name="cw", bufs=1)
        with tc.tile_pool(name="st2", bufs=2) as stp2:
          with tc.tile_pool(name="pc1", bufs=2, space="PSUM") as pc1:
              if not zb:
                  b2r = wpool.tile([1, E, D], BF16)
                  nc.sync.dma_start(b2r[:], d_b2r.ap()[:])
              # e2/e3 expert weights (gpsimd queue blocks on WAR until
              # e0/e1 are consumed mid phase D -- gpsimd is idle then)
              _w_issue(2, nc.gpsimd)
              _w_issue(3, nc.gpsimd)
              x_sb = wpool.tile([128, NTT, D], F32R)      # post-LN1
              xT32 = wpool.tile([128, DC, NTT, 128], F32R)  # x^T f32 (gate)
              xT8 = wpool.tile([128, DC, NTT, 128], FP8)  # x^T * 16 (fp8)
              comb = wpool.tile([128, NTT, E], F32)       # top-2 combine weights
              ffs = wpool.tile([128, NTT, D], F32)
              out_sb = wpool.tile([128, NTT, D], F32)

              def layer_norm(dst_ap, pre_ap, gb_idx):
                  s1 = stp2.tile([128, 1], F32, tag="s1")
                  q1 = stp2.tile([128, 1], F32, tag="q1")
                  mn = stp2.tile([128, 1], F32, tag="mn")
                  vv = stp2.tile([128, 1], F32, tag="vv")
                  rs = stp2.tile([128, 1], F32, tag="rs")
                  bb = stp2.tile([128, 1], F32, tag="bb")
                  xn = stp2.tile([128, D], F32, tag="xn")
                  sq2 = stp2.tile([128, D], F32, tag="xn")
                  nc.vector.reduce_sum(s1[:], pre_ap, axis=AX.X)
                  nc.scalar.activation(sq2[:], pre_ap, AF.Square, accum_out=q1[:])
                  nc.vector.tensor_scalar_mul(mn[:], s1[:], 1.0 / D)
                  nc.vector.tensor_tensor(vv[:], mn[:], mn[:], op=ALU.mult)
                  nc.vector.scalar_tensor_tensor(vv[:], q1[:], 1.0 / D, vv[:],
                                                 op0=ALU.mult, op1=ALU.subtract)
                  sr = stp2.tile([128, 1], F32, tag="sr")
                  nc.scalar.activation(sr[:], vv[:], AF.Sqrt, bias=epsc[:])
                  nc.vector.reciprocal(rs[:], sr[:])
                  nc.vector.scalar_tensor_tensor(bb[:], mn[:], -1.0, rs[:],
                                                 op0=ALU.mult, op1=ALU.mult)
                  if zb:
                      nc.scalar.activation(dst_ap, pre_ap, AF.Identity,
                                           bias=bb[:], scale=rs[:])
                  else:
                      nc.scalar.activation(xn[:], pre_ap, AF.Identity,
                                           bias=bb[:], scale=rs[:])
                      nc.vector.tensor_tensor(xn[:], xn[:],
                                              lnbc[:, 2 * gb_idx, :], op=ALU.mult)
                      nc.vector.tensor_tensor(dst_ap, xn[:],
                                              lnbc[:, 2 * gb_idx + 1, :],
                                              op=ALU.add)

              for tt in range(NTT):
                  ps = pc1.tile([128, D], F32, tag="wo")
                  for (cs, ce) in chunks:
                      for c in range(DC):
                          nc.tensor.matmul(ps[:, cs:ce],
                                           attnT[:, c, tt * 128:(tt + 1) * 128],
                                           wo[:, c, cs:ce], start=(c == 0),
                                           stop=(c == DC - 1))
                  pre = stp2.tile([128, D], F32, tag="pre")
                  nc.vector.tensor_tensor(pre[:], ps[:], srcq[:, tt, :], op=ALU.add)
                  layer_norm(x_sb[:, tt, :], pre[:], 0)

              # x^T
              for c in range(DC):
                  xtp = pc1.tile([128, NTT, 128], F32R, tag="xtp")
                  for tt in range(NTT):
                      nc.tensor.transpose(xtp[:, tt, :],
                                          x_sb[:, tt, c * 128:(c + 1) * 128], identr[:])
                  nc.vector.tensor_copy(xT32[:, c], xtp[:])
                  nc.scalar.mul(xT8[:, c], xtp[:], 16.0)

              # gate + top-2 combine
              for tt in range(NTT):
                  gp = pc1.tile([128, E], F32, tag="gate")
                  for c in range(DC):
                      nc.tensor.matmul(gp[:], xT32[:, c, tt, :], wg[:, c, :],
                                       start=(c == 0),
                                       stop=(zb and c == DC - 1))
                  if not zb:
                      nc.tensor.matmul(gp[:], ones1[0:1, :], bgr[0:1, :],
                                       start=False, stop=True)
                  mx = stp2.tile([128, 1], F32, tag="mx")
                  se = stp2.tile([128, 1], F32, tag="se")
                  eg = stp2.tile([128, E], F32, tag="eg")
                  pr = stp2.tile([128, E], F32, tag="pr")
                  m2 = stp2.tile([128, 1], F32, tag="m2")
                  kp = stp2.tile([128, E], F32, tag="kp")
                  nc.vector.reduce_max(mx[:], gp[:], axis=AX.X)
                  nc.vector.tensor_scalar_mul(mx[:], mx[:], -1.0)
                  nc.scalar.activation(eg[:], gp[:], AF.Exp, bias=mx[:], accum_out=se[:])
                  nc.vector.reciprocal(se[:], se[:])
                  nc.vector.tensor_scalar_mul(pr[:], eg[:], se[:])
                  nc.vector.reduce_max(mx[:], pr[:], axis=AX.X)
                  nc.vector.tensor_scalar(kp[:], pr[:], mx[:], None, op0=ALU.is_ge)
                  nc.vector.scalar_tensor_tensor(eg[:], kp[:], -1e9, pr[:],
                                                 op0=ALU.mult, op1=ALU.add)
                  nc.vector.reduce_max(m2[:], eg[:], axis=AX.X)
                  nc.vector.tensor_scalar(kp[:], pr[:], m2[:], None, op0=ALU.is_ge)
                  nc.vector.tensor_tensor(comb[:, tt, :], pr[:], kp[:], op=ALU.mult)
                  nc.vector.tensor_scalar(comb[:, tt, :], comb[:, tt, :],
                                          2.0 ** -14, None, op0=ALU.mult)

          # ============ Phase D: MoE experts ============
          with tc.tile_pool(name="mh", bufs=3) as mhp, \
               tc.tile_pool(name="pd1", bufs=3, space="PSUM") as pd1, \
               tc.tile_pool(name="pd2", bufs=1, space="PSUM") as pd2:
              for e in range(E):
                  w1t, w2t = west[e]
                  yps = [pd2.tile([128, D], F32, tag=f"y{tt}", name=f"y{tt}") for tt in range(NTT)]
                  for fp in range(FP):
                      hp = pd1.tile([128, 2, QTOK], F32, tag="hps")
                      for t in range(2):
                          for cp in range(3):
                              nc.tensor.matmul(hp[:, t, :], w1t[:, fp, t, cp],
                                               xT8[:, 2 * cp:2 * cp + 2]
                                               .rearrange("p c t f -> p c (t f)"),
                                               start=(cp == 0), stop=(cp == 2),
                                               perf_mode=PM.DoubleRow)
                      hsb = mhp.tile([128, 2, QTOK], FP8, tag="hsb")
                      if fp % 2 == 0:
                          nc.scalar.activation(hsb[:], hp[:], AF.Relu,
                                               scale=2.0 ** -6)
                      else:
                          nc.vector.tensor_scalar(hsb[:], hp[:], 2.0 ** -6,
                                                  0.0, op0=ALU.mult,
                                                  op1=ALU.max)
                      for tt in range(NTT):
                          for (cs, ce) in chunks:
                              nc.tensor.matmul(yps[tt][:, cs:ce],
                                               hsb[:, :, tt * 128:(tt + 1) * 128],
                                               w2t[:, fp, :, cs:ce],
                                               start=(fp == 0),
                                               stop=(zb and fp == FP - 1),
                                               perf_mode=PM.DoubleRow)
                  for tt in range(NTT):
                      if not zb:
                          for (cs, ce) in chunks:
                              nc.tensor.matmul(yps[tt][:, cs:ce], ones1b[0:1, :],
                                               b2r[0:1, e, cs:ce],
                                               start=False, stop=True)
                      if e == 0:
                          nc.scalar.mul(ffs[:, tt, :], yps[tt][:],
                                        comb[:, tt, e:e + 1])
                      else:
                          nc.vector.scalar_tensor_tensor(
                              ffs[:, tt, :], yps[tt][:], comb[:, tt, e:e + 1],
                              ffs[:, tt, :], op0=ALU.mult, op1=ALU.add)

              for tt in range(NTT):
                  pre2 = stp2.tile([128, D], F32, tag="pre")
                  nc.vector.tensor_tensor(pre2[:], x_sb[:, tt, :], ffs[:, tt, :],
                                          op=ALU.add)
                  layer_norm(out_sb[:, tt, :], pre2[:], 1)
              for tt in range(NTT):
                  nc.sync.dma_start(
                      d_out.ap()[tt * 128:(tt + 1) * 128].rearrange(
                          "(o p) d -> p o d", p=128), out_sb[:, tt:tt + 1, :])
        wpool.release()
        perpool.release()
        mwp0.release()
        bpool.release()
        cpool.release()

    nc.compile()
    return nc


def _prep(inputs):
    f = lambda a: np.ascontiguousarray(np.asarray(a, dtype=np.float32))
    bf = lambda a: np.ascontiguousarray(a).astype(ml_dtypes.bfloat16)
    src = f(inputs["src"])
    scale = (D // H) ** -0.5
    lnrows = np.concatenate([
        f(inputs["ln1_g"]), f(inputs["ln1_b"]),
        f(inputs["ln2_g"]), f(inputs["ln2_b"]),
        f(inputs["bv"]), f(inputs["bo"])]).reshape(1, 6 * D)
    common = {
        "wqT": bf(f(inputs["Wq"]).T * scale),
        "wkT": bf(f(inputs["Wk"]).T),
        "wvT": bf(f(inputs["Wv"]).T),
        "woT": bf(f(inputs["Wo"]).T),
        "bqc": (f(inputs["bq"]) * scale).reshape(DC, 128).T.copy(),
        "bkc": f(inputs["bk"]).reshape(DC, 128).T.copy(),
        "lnrows": bf(lnrows),
        "wgT": np.ascontiguousarray(f(inputs["Wg"]).T),
        "bgr": f(inputs["bg"]).reshape(1, E),
        "w1": np.ascontiguousarray(
            (f(inputs["W1"]) * 256.0).reshape(E, 3, 2, 128, FP, 2, 128)
            .transpose(0, 3, 4, 5, 1, 2, 6)).astype(np.dtype("float8_e4m3")),
        "w2": np.ascontiguousarray(
            (f(inputs["W2"]) * 256.0).reshape(E, FP, 2, 128, D)
            .transpose(0, 3, 1, 2, 4)).astype(np.dtype("float8_e4m3")),
        "b2r": bf(f(inputs["b2"]).reshape(1, E, D) * (2.0 ** 14)),
        "ident": np.eye(128, dtype=np.float32),
        "ones_r": np.ones((1, 128), dtype=np.float32),
    }
    assert not np.any(f(inputs["b1"])), "fp8 MoE path requires zero b1"
    in_maps = []
    for c in range(NCORES):
        b, qq = c // 4, c % 4
        m = dict(common)
        # rotate key/value token axis so this core's quarter sits at cols 0:256
        m["srcT"] = bf(np.roll(src[b].T, -qq * QTOK, axis=1))
        m["srcq"] = np.ascontiguousarray(src[b, qq * QTOK:(qq + 1) * QTOK])
        in_maps.append(m)
    return in_maps


def kernel(**inputs):
    global LAST_RESULT
    gamma = float(np.asarray(inputs["gamma"]))
    zb = (not any(np.any(np.asarray(inputs[k])) for k in
                  ("bq", "bk", "bv", "bo", "bg", "b2", "ln1_b", "ln2_b"))
          and all(np.all(np.asarray(inputs[k]) == 1.0) for k in ("ln1_g", "ln2_g")))
    key = (round(gamma, 9), zb)
    if key not in _cache:
        _cache[key] = _build(gamma, zb)
    nc = _cache[key]
    in_maps = _prep(inputs)
    trace = bool(os.environ.get("KERNEL_TRACE"))
    try:
        res = run_bass_kernel_spmd(nc, in_maps, list(range(NCORES)), trace=trace)
    except ModuleNotFoundError:
        res = run_bass_kernel_spmd(nc, in_maps, list(range(NCORES)), trace=False)
    LAST_RESULT = res
    out = np.empty((B, T, D), dtype=np.float32)
    for c in range(NCORES):
        b, qq = c // 4, c % 4
        out[b, qq * QTOK:(qq + 1) * QTOK] = res.results[c]["out"]
    return out


# revision 29
# speedup vs baseline: 1.0585x; 1.0431x over previous
"""Trainium2 Bass kernel for CustomTransformerEncoderMoELayer (moe_routing).

Sharding: 8 cores = 2 batches x 4 query-quarters. Each core:
  - projects Q^T for its 256 query tokens, K^T/V for its full batch
    (replicated within batch group),
  - z-score softmax attention with matmul-derived row statistics
    (sum S via q.Kbar, sum S^2 via q^T M q, M = K^T K per head),
  - residual + LN1, dense 4-expert MoE (fp8 DoubleRow), residual + LN2.
No cross-core communication; host only shards inputs / concatenates outputs.
"""
import os
import numpy as np
import ml_dtypes

import concourse.bacc as bacc
import concourse.mybir as mybir
import concourse.tile as tile
from concourse.bass_utils import run_bass_kernel_spmd

F32 = mybir.dt.float32
F32R = mybir.dt.float32r
BF16 = mybir.dt.bfloat16
FP8 = mybir.dt.float8e4
PM = mybir.MatmulPerfMode
AF = mybir.ActivationFunctionType
ALU = mybir.AluOpType
AX = mybir.AxisListType

B, T, D, FFD, E, H = 2, 1024, 768, 3072, 4, 12
HD = D // H          # 64
QTOK = 256           # query tokens per core
NCORES = 8
DC = D // 128        # 6 chunks of contraction dim
FT = FFD // 128      # 24 FF tiles
FP = FFD // 256      # 12 fp8 DoubleRow units per expert
NTT = QTOK // 128    # 2 token tiles
KB = T // 128        # 8 key blocks
EPS = 1e-5

_cache = {}
LAST_RESULT = None


def _build(gamma: float, zb: bool):
    """zb: zero-bias/unit-gain fast path (all attn/MoE biases 0, LN gains 1)."""
    nc = bacc.Bacc("TRN2", target_bir_lowering=False, debug=False,
                   num_devices=NCORES)

    # ---- DRAM I/O ----
    d_srcT = nc.dram_tensor("srcT", [D, T], BF16, kind="ExternalInput")
    d_srcq = nc.dram_tensor("srcq", [QTOK, D], F32, kind="ExternalInput")
    d_wqT = nc.dram_tensor("wqT", [D, D], BF16, kind="ExternalInput")
    d_wkT = nc.dram_tensor("wkT", [D, D], FP8, kind="ExternalInput")
    d_wvT = nc.dram_tensor("wvT", [D, D], FP8, kind="ExternalInput")
    d_woT = nc.dram_tensor("woT", [D, D], BF16, kind="ExternalInput")
    d_bqc = nc.dram_tensor("bqc", [128, DC], F32, kind="ExternalInput")
    d_bkc = nc.dram_tensor("bkc", [128, DC], F32, kind="ExternalInput")
    d_ln = nc.dram_tensor("lnrows", [1, 6 * D], BF16, kind="ExternalInput")
    d_wgT = nc.dram_tensor("wgT", [D, E], F32R, kind="ExternalInput")
    d_bgr = nc.dram_tensor("bgr", [1, E], F32R, kind="ExternalInput")
    d_w1 = nc.dram_tensor("w1", [E, 128, FP, 2, 3, 2, 128], FP8,
                          kind="ExternalInput")
    d_w2 = nc.dram_tensor("w2", [E, 128, FP, 2, D], FP8, kind="ExternalInput")
    d_b2r = nc.dram_tensor("b2r", [1, E, D], BF16, kind="ExternalInput")
    d_ident = nc.dram_tensor("ident", [128, 128], F32R, kind="ExternalInput")
    d_ones = nc.dram_tensor("ones_r", [1, 128], F32R, kind="ExternalInput")
    d_out = nc.dram_tensor("out", [QTOK, D], F32, kind="ExternalOutput")

    chunks = [(0, 512), (512, 768)]  # free-dim chunks of D for matmul N<=512

    with tile.TileContext(nc) as tc:
        # ----- persistent pools -----
        cpool = tc.alloc_tile_pool(name="const", bufs=1)
        identr = cpool.tile([128, 128], F32R)
        nc.sync.dma_start(identr[:], d_ident.ap()[:])
        identb = cpool.tile([128, 128], BF16)
        nc.vector.tensor_copy(identb[:], identr[:])
        ones1 = cpool.tile([1, 128], F32R)
        nc.sync.dma_start(ones1[:], d_ones.ap()[:])
        ones2 = cpool.tile([128, 2], BF16)   # col0 = 0, col1 = 1
        nc.vector.memset(ones2[:], 0.0)
        nc.vector.memset(ones2[:, 1:2], 1.0)
        ones1b = cpool.tile([1, 128], BF16)
        nc.vector.memset(ones1b[:], 1.0)
        zerob = cpool.tile([1, 1], BF16)
        nc.vector.memset(zerob[:], 0.0)
        bqc = cpool.tile([128, DC], F32)
        nc.sync.dma_start(bqc[:], d_bqc.ap()[:])
        bkc = cpool.tile([128, DC], F32)
        nc.sync.dma_start(bkc[:], d_bkc.ap()[:])
        bgr = cpool.tile([1, E], F32R)
        nc.sync.dma_start(bgr[:], d_bgr.ap()[:])
        epsc = cpool.tile([128, 1], F32)
        nc.vector.memset(epsc[:], EPS)
        # rows: ln1g ln1b ln2g ln2b bv bo, broadcast across partitions
        lnbc = cpool.tile([128, 6, D], BF16)
        lnp = tc.alloc_tile_pool(name="lnp", bufs=1)
        lnrow = lnp.tile([1, 6 * D], BF16)
        nc.sync.dma_start(lnrow[:], d_ln.ap()[:])
        for i in range(6):
            nc.gpsimd.partition_broadcast(lnbc[:, i, :],
                                          lnrow[0:1, i * D:(i + 1) * D])
        lnp.release()

        bpool = tc.alloc_tile_pool(name="bp", bufs=1)
        attnT = bpool.tile([128, DC, QTOK], BF16)     # attn^T (normalized)
        srcq = bpool.tile([128, NTT, D], F32)         # becomes srcq + bo
        wo = bpool.tile([128, DC, D], BF16)
        wg = bpool.tile([128, DC, E], F32R)

        # MoE expert weights: one DMA per (expert, matrix), ping-pong 2 experts
        mwp0 = tc.alloc_tile_pool(name="mw0", bufs=2)
        west = {}

        def _w_issue(e, eng):
            w1t = mwp0.tile([128, FP, 2, 3, 2, 128], FP8, tag="w1t",
                            name=f"w1e{e}")
            eng.dma_start(w1t[:], d_w1.ap()[e])
            w2t = mwp0.tile([128, FP, 2, D], FP8, tag="w2t", name=f"w2e{e}")
            eng.dma_start(w2t[:], d_w2.ap()[e])
            west[e] = (w1t, w2t)

        perpool = tc.alloc_tile_pool(name="per", bufs=1)
        qT = perpool.tile([128, DC, QTOK], BF16)      # Q^T * scale (+bias)
        kT = perpool.tile([128, DC, T], BF16)         # K^T
        vN = perpool.tile([128, KB, D], BF16)         # V natural [t,d]
        kbar = perpool.tile([128, DC, 1], F32)        # row sums of kT
        kb2 = perpool.tile([128, H, 2], BF16)         # col0 = kbar_h, col1 = 0
        sexp = perpool.tile([128, NTT, H], F32)
        bexp = perpool.tile([128, NTT, H], F32)

        # ================= Phase A: projections =================
        with tc.tile_pool(name="aw", bufs=1) as awp, \
             tc.tile_pool(name="pa1", bufs=3, space="PSUM") as pa1, \
             tc.tile_pool(name="pa2", bufs=2, space="PSUM") as pa2:
            wq = awp.tile([128, DC, D], BF16)
            nc.sync.dma_start(wq[:], d_wqT.ap().rearrange("(c p) d -> p c d", p=128))
            srcT = awp.tile([128, DC, T], BF16)
            nc.sync.dma_start(srcT[:, :, 0:QTOK],
                              d_srcT.ap()[:, 0:QTOK].rearrange("(c p) t -> p c t", p=128))
            nc.sync.dma_start(srcT[:, :, QTOK:T],
                              d_srcT.ap()[:, QTOK:T].rearrange("(c p) t -> p c t", p=128))
            wk = awp.tile([128, DC, D], FP8)
            nc.sync.dma_start(wk[:], d_wkT.ap().rearrange("(c p) d -> p c d", p=128))
            wv = awp.tile([128, DC, D], FP8)
            nc.sync.dma_start(wv[:], d_wvT.ap().rearrange("(c p) d -> p c d", p=128))
            srcT8 = awp.tile([128, DC, T], FP8)   # fp8 copy for K/V DR matmuls
            nc.sync.dma_start(srcq[:],
                              d_srcq.ap().rearrange("(tt p) d -> p tt d", p=128))
            nc.sync.dma_start(wo[:], d_woT.ap().rearrange("(c p) d -> p c d", p=128))
            nc.sync.dma_start(wg[:], d_wgT.ap().rearrange("(c p) e -> p c e", p=128))

            # Q^T [D, 256] (pre-scaled by 1/sqrt(hd) on host, incl bias)
            for m in range(DC):
                ps = pa1.tile([128, 512], F32, tag="prj")
                for c in range(DC):
                    nc.tensor.matmul(ps[:, 0:QTOK], wq[:, c, m * 128:(m + 1) * 128],
                                     srcT[:, c, 0:QTOK], start=(c == 0), stop=(c == DC - 1))
                if zb:
                    nc.scalar.copy(qT[:, m, :], ps[:, 0:QTOK])
                else:
                    nc.scalar.activation(qT[:, m, :], ps[:, 0:QTOK], AF.Identity,
                                         bias=bqc[:, m:m + 1])
            # fp8 source copy (weights carry x16; results rescaled by 2^-4)
            nc.vector.tensor_copy(srcT8[:], srcT[:])
            # K^T [D, 1024] via fp8 DoubleRow over d-chunk pairs
            for m in range(DC):
                for kc in range(2):
                    ps = pa1.tile([128, 512], F32, tag="prj")
                    for cp in range(3):
                        nc.tensor.matmul(ps[:],
                                         wk[:, 2 * cp:2 * cp + 2,
                                            m * 128:(m + 1) * 128],
                                         srcT8[:, 2 * cp:2 * cp + 2,
                                               kc * 512:(kc + 1) * 512],
                                         start=(cp == 0), stop=(cp == 2),
                                         perf_mode=PM.DoubleRow)
                    if zb:
                        nc.scalar.activation(kT[:, m, kc * 512:(kc + 1) * 512],
                                             ps[:], AF.Identity, scale=2.0 ** -4)
                    else:
                        nc.scalar.activation(kT[:, m, kc * 512:(kc + 1) * 512],
                                             ps[:], AF.Identity, scale=2.0 ** -4,
                                             bias=bkc[:, m:m + 1])
            # V natural [T, D] via fp8 DoubleRow; bv added via broadcast row
            for t8 in range(KB):
                ps = pa2.tile([128, D], F32, tag="vprj")
                for (cs, ce) in chunks:
                    for cp in range(3):
                        nc.tensor.matmul(ps[:, cs:ce],
                                         srcT8[:, 2 * cp:2 * cp + 2,
                                               t8 * 128:(t8 + 1) * 128],
                                         wv[:, 2 * cp:2 * cp + 2, cs:ce],
                                         start=(cp == 0), stop=(cp == 2),
                                         perf_mode=PM.DoubleRow)
                if zb:
                    nc.vector.tensor_scalar(vN[:, t8, :], ps[:], 2.0 ** -4,
                                            None, op0=ALU.mult)
                else:
                    nc.vector.scalar_tensor_tensor(vN[:, t8, :], ps[:], 2.0 ** -4,
                                                   lnbc[:, 4, :],
                                                   op0=ALU.mult, op1=ALU.add)
            # row-sums of kT for Sum(S) stats
            nc.vector.reduce_sum(kbar[:], kT[:], axis=AX.X)
            nc.vector.memset(kb2[:], 0.0)
            for h in range(H):
                th, off = (64 * h) // 128, (64 * h) % 128
                nc.vector.tensor_copy(kb2[off:off + 64, h, 0:1],
                                      kbar[off:off + 64, th, :])
            if not zb:  # srcq + bo (used post-Wo)
                for tt in range(NTT):
                    nc.vector.tensor_tensor(srcq[:, tt, :], srcq[:, tt, :],
                                            lnbc[:, 5, :], op=ALU.add)

        # ======== Phase B0: z-score statistics via K moments ========
        # SumS[q]  = q . Kbar         (per head)
        # SumS2[q] = q^T (K^T K) q    (per head)
        spool = tc.alloc_tile_pool(name="sp", bufs=3)
        statp = tc.alloc_tile_pool(name="stat", bufs=1)
        _w_issue(0, nc.gpsimd)
        _w_issue(1, nc.gpsimd)
        with tc.tile_pool(name="kn", bufs=2) as knp, \
             tc.tile_pool(name="pstk", bufs=2, space="PSUM") as pstk, \
             tc.tile_pool(name="pst1", bufs=1, space="PSUM") as pst1, \
             tc.tile_pool(name="stw", bufs=2) as stw:
            st_sb = statp.tile([2, H, QTOK], F32R)     # [stat, head, q]
            for h in range(H):
                th, off = (64 * h) // 128, (64 * h) % 128
                kn = pstk.tile([128, KB, 64], BF16, tag="kn")
                for kb in range(KB):
                    nc.tensor.transpose(kn[:, kb, :],
                                        kT[off:off + 64, th, kb * 128:(kb + 1) * 128],
                                        identb[off:off + 64, off:off + 64])
                knb = knp.tile([128, KB, 64], BF16, tag="knb")
                nc.scalar.copy(knb[:], kn[:])
                mps = pst1.tile([64, 64], F32, tag="mps")
                for kb in range(KB):
                    nc.tensor.matmul(mps[:], knb[:, kb, :], knb[:, kb, :],
                                     start=(kb == 0), stop=(kb == KB - 1))
                msb = stw.tile([128, 64], BF16, tag="msb")
                nc.vector.tensor_copy(msb[off:off + 64, :], mps[:])
                rps = pst1.tile([64, QTOK], F32, tag="rps")
                nc.tensor.matmul(rps[:], msb[off:off + 64, :],
                                 qT[off:off + 64, th, :], start=True, stop=True)
                tpr = stw.tile([128, QTOK], BF16, tag="tpr")
                nc.vector.tensor_tensor(tpr[off:off + 64, :], rps[:],
                                        qT[off:off + 64, th, :], op=ALU.mult)
                pstat = pst1.tile([2, QTOK], F32, tag="sst")
                nc.tensor.matmul(pstat[:], kb2[off:off + 64, h, :],
                                 qT[off:off + 64, th, :], start=True, stop=False)
                nc.tensor.matmul(pstat[:], ones2[off:off + 64, :],
                                 tpr[off:off + 64, :], start=False, stop=True)
                nc.vector.tensor_copy(st_sb[:, h, :], pstat[:])

        # ================= Phase B2: attention =================
        # Software pipelined: S matmuls run one head ahead, PV one head
        # behind, so the tensor queue never waits on the exp/P2/copy chain.
        with tc.tile_pool(name="pba", bufs=1, space="PSUM") as pba, \
             tc.tile_pool(name="pbst", bufs=1, space="PSUM") as pbst, \
             tc.tile_pool(name="pbt", bufs=2, space="PSUM") as pbt, \
             tc.tile_pool(name="pb1", bufs=2, space="PSUM") as pbs, \
             tc.tile_pool(name="st2b", bufs=2) as stp2b:
            Sq = {}
            PTq = {}

            def emit_S(h):
                th, off = (64 * h) // 128, (64 * h) % 128
                for qt in range(NTT):
                    S = pbs.tile([128, T], F32, tag="S")
                    for kc in range(2):
                        nc.tensor.matmul(
                            S[:, kc * 512:(kc + 1) * 512],
                            qT[off:off + 64, th, qt * 128:(qt + 1) * 128],
                            kT[off:off + 64, th, kc * 512:(kc + 1) * 512],
                            start=True, stop=True)
                    Sq[(h, qt)] = S

            def emit_soft(h):
                # exp + normalize + transpose + copy for head h
                den = stp2b.tile([128, NTT], F32, tag="den")
                rden = stp2b.tile([128, NTT], F32, tag="rden")
                PTsb = spool.tile([128, NTT, KB, 128], BF16, tag="PTsb")
                for qt in range(NTT):
                    S = Sq.pop((h, qt))
                    P = spool.tile([128, T], BF16, tag="P")
                    nc.scalar.activation(P[:], S[:], AF.Exp,
                                         bias=bexp[:, qt, h:h + 1],
                                         scale=sexp[:, qt, h:h + 1],
                                         accum_out=den[:, qt:qt + 1])
                    nc.vector.reciprocal(rden[:, qt:qt + 1], den[:, qt:qt + 1])
                    P2 = spool.tile([128, T], BF16, tag="P2")
                    nc.vector.tensor_scalar(P2[:], P[:], rden[:, qt:qt + 1],
                                            None, op0=ALU.mult)
                    PT = pbt.tile([128, KB, 128], BF16, tag="PT")
                    for kb in range(KB):
                        nc.tensor.transpose(PT[:, kb, :],
                                            P2[:, kb * 128:(kb + 1) * 128],
                                            identb[:])
                    nc.vector.tensor_copy(PTsb[:, qt], PT[:])
                PTq[h] = PTsb

            def emit_pv(h):
                th, off = (64 * h) // 128, (64 * h) % 128
                PTsb = PTq.pop(h)
                aps = pba.tile([64, QTOK], F32, tag="aps")
                for kb in range(KB):
                    nc.tensor.matmul(aps[:], vN[:, kb, h * 64:(h + 1) * 64],
                                     PTsb[:, :, kb, :], start=(kb == 0),
                                     stop=(kb == KB - 1))
                nc.vector.tensor_copy(attnT[off:off + 64, th, :], aps[:])

            emit_S(0)
            # stats finalize: transpose to [q, (h, stat)], exp scale/bias
            stT = statp.tile([128, NTT, H, 2], F32)
            for qt in range(NTT):
                stps = pbst.tile([128, H, 2], F32R, tag="stps")
                for h in range(H):
                    nc.tensor.transpose(stps[:, h, :],
                                        st_sb[:, h, qt * 128:(qt + 1) * 128],
                                        identr[0:2, 0:2])
                nc.vector.tensor_copy(stT[:, qt], stps[:])
            for qt in range(NTT):
                sums = stT[:, qt, :, 0]
                sqs = stT[:, qt, :, 1]
                mean = stp2b.tile([128, H], F32, tag="mean")
                tm1 = stp2b.tile([128, H], F32, tag="tm1")
                sig = stp2b.tile([128, H], F32, tag="sig")
                nc.vector.tensor_scalar_mul(mean[:], sums, 1.0 / T)
                nc.vector.tensor_tensor(tm1[:], sums, mean[:], op=ALU.mult)
                nc.vector.tensor_sub(tm1[:], sqs, tm1[:])
                nc.scalar.activation(sig[:], tm1[:], AF.Sqrt, scale=1.0 / (T - 1))
                nc.vector.tensor_scalar_add(sig[:], sig[:], EPS)
                nc.vector.reciprocal(sexp[:, qt, :], sig[:])
                if gamma != 1.0:
                    nc.vector.tensor_scalar_mul(sexp[:, qt, :], sexp[:, qt, :],
                                                float(gamma))
                nc.vector.scalar_tensor_tensor(bexp[:, qt, :], mean[:], -1.0,
                                               sexp[:, qt, :],
                                               op0=ALU.mult, op1=ALU.mult)
            for h in range(H):
                if h + 1 < H:
                    emit_S(h + 1)
                emit_soft(h)
                if h >= 1:
                    emit_pv(h - 1)
            emit_pv(H - 1)
        statp.release()
        spool.release()

        # ============ Phase C: Wo + LN1 + x^T + gate ============
        wpool = tc.alloc_tile_pool(name="cw", bufs=1)
        with tc.tile_pool(name="st2", bufs=2) as stp2:
          with tc.tile_pool(name="pc1", bufs=2, space="PSUM") as pc1:
              if not zb:
                  b2r = wpool.tile([1, E, D], BF16)
                  nc.sync.dma_start(b2r[:], d_b2r.ap()[:])
              # e2/e3 expert weights (gpsimd queue blocks on WAR until
              # e0/e1 are consumed mid phase D -- gpsimd is idle then)
              _w_issue(2, nc.gpsimd)
              _w_issue(3, nc.gpsimd)
              x_sb = wpool.tile([128, NTT, D], F32R)      # post-LN1
              xT32 = wpool.tile([128, DC, NTT, 128], F32R)  # x^T f32 (gate)
              xT8 = wpool.tile([128, DC, NTT, 128], FP8)  # x^T * 16 (fp8)
              comb = wpool.tile([128, NTT, E], F32)       # top-2 combine weights
              ffs = wpool.tile([128, NTT, D], F32)
              out_sb = wpool.tile([128, NTT, D], F32)

              def layer_norm(dst_ap, pre_ap, gb_idx):
                  s1 = stp2.tile([128, 1], F32, tag="s1")
                  q1 = stp2.tile([128, 1], F32, tag="q1")
                  mn = stp2.tile([128, 1], F32, tag="mn")
                  vv = stp2.tile([128, 1], F32, tag="vv")
                  rs = stp2.tile([128, 1], F32, tag="rs")
                  bb = stp2.tile([128, 1], F32, tag="bb")
                  xn = stp2.tile([128, D], F32, tag="xn")
                  sq2 = stp2.tile([128, D], F32, tag="xn")
                  nc.vector.reduce_sum(s1[:], pre_ap, axis=AX.X)
                  nc.scalar.activation(sq2[:], pre_ap, AF.Square, accum_out=q1[:])
                  nc.vector.tensor_scalar_mul(mn[:], s1[:], 1.0 / D)
                  nc.vector.tensor_tensor(vv[:], mn[:], mn[:], op=ALU.mult)
                  nc.vector.scalar_tensor_tensor(vv[:], q1[:], 1.0 / D, vv[:],
                                                 op0=ALU.mult, op1=ALU.subtract)
                  sr = stp2.tile([128, 1], F32, tag="sr")
                  nc.scalar.activation(sr[:], vv[:], AF.Sqrt, bias=epsc[:])
                  nc.vector.reciprocal(rs[:], sr[:])
                  nc.vector.scalar_tensor_tensor(bb[:], mn[:], -1.0, rs[:],
                                                 op0=ALU.mult, op1=ALU.mult)
                  if zb:
                      nc.scalar.activation(dst_ap, pre_ap, AF.Identity,
                                           bias=bb[:], scale=rs[:])
                  else:
                      nc.scalar.activation(xn[:], pre_ap, AF.Identity,
                                           bias=bb[:], scale=rs[:])
                      nc.vector.tensor_tensor(xn[:], xn[:],
                                              lnbc[:, 2 * gb_idx, :], op=ALU.mult)
                      nc.vector.tensor_tensor(dst_ap, xn[:],
                                              lnbc[:, 2 * gb_idx + 1, :],
                                              op=ALU.add)

              for tt in range(NTT):
                  ps = pc1.tile([128, D], F32, tag="wo")
                  for (cs, ce) in chunks:
                      for c in range(DC):
                          nc.tensor.matmul(ps[:, cs:ce],
                                           attnT[:, c, tt * 128:(tt + 1) * 128],
                                           wo[:, c, cs:ce], start=(c == 0),
                                           stop=(c == DC - 1))
                  pre = stp2.tile([128, D], F32, tag="pre")
                  nc.vector.tensor_tensor(pre[:], ps[:], srcq[:, tt, :], op=ALU.add)
                  layer_norm(x_sb[:, tt, :], pre[:], 0)

              # x^T
              for c in range(DC):
                  xtp = pc1.tile([128, NTT, 128], F32R, tag="xtp")
                  for tt in range(NTT):
                      nc.tensor.transpose(xtp[:, tt, :],
                                          x_sb[:, tt, c * 128:(c + 1) * 128], identr[:])
                  nc.vector.tensor_copy(xT32[:, c], xtp[:])
                  nc.scalar.mul(xT8[:, c], xtp[:], 16.0)

              # gate + top-2 combine
              for tt in range(NTT):
                  gp = pc1.tile([128, E], F32, tag="gate")
                  for c in range(DC):
                      nc.tensor.matmul(gp[:], xT32[:, c, tt, :], wg[:, c, :],
                                       start=(c == 0),
                                       stop=(zb and c == DC - 1))
                  if not zb:
                      nc.tensor.matmul(gp[:], ones1[0:1, :], bgr[0:1, :],
                                       start=False, stop=True)
                  mx = stp2.tile([128, 1], F32, tag="mx")
                  se = stp2.tile([128, 1], F32, tag="se")
                  eg = stp2.tile([128, E], F32, tag="eg")
                  pr = stp2.tile([128, E], F32, tag="pr")
                  m2 = stp2.tile([128, 1], F32, tag="m2")
                  kp = stp2.tile([128, E], F32, tag="kp")
                  nc.vector.reduce_max(mx[:], gp[:], axis=AX.X)
                  nc.vector.tensor_scalar_mul(mx[:], mx[:], -1.0)
                  nc.scalar.activation(eg[:], gp[:], AF.Exp, bias=mx[:], accum_out=se[:])
                  nc.vector.reciprocal(se[:], se[:])
                  nc.vector.tensor_scalar_mul(pr[:], eg[:], se[:])
                  nc.vector.reduce_max(mx[:], pr[:], axis=AX.X)
                  nc.vector.tensor_scalar(kp[:], pr[:], mx[:], None, op0=ALU.is_ge)
                  nc.vector.scalar_tensor_tensor(eg[:], kp[:], -1e9, pr[:],
                                                 op0=ALU.mult, op1=ALU.add)
                  nc.vector.reduce_max(m2[:], eg[:], axis=AX.X)
                  nc.vector.tensor_scalar(kp[:], pr[:], m2[:], None, op0=ALU.is_ge)
                  nc.vector.tensor_tensor(comb[:, tt, :], pr[:], kp[:], op=ALU.mult)
                  nc.vector.tensor_scalar(comb[:, tt, :], comb[:, tt, :],
                                          2.0 ** -14, None, op0=ALU.mult)

          # ============ Phase D: MoE experts ============
          with tc.tile_pool(name="mh", bufs=3) as mhp, \
               tc.tile_pool(name="pd1", bufs=3, space="PSUM") as pd1, \
               tc.tile_pool(name="pd2", bufs=1, space="PSUM") as pd2:
              for e in range(E):
                  w1t, w2t = west[e]
                  yps = [pd2.tile([128, D], F32, tag=f"y{tt}", name=f"y{tt}") for tt in range(NTT)]
                  for fp in range(FP):
                      hp = pd1.tile([128, 2, QTOK], F32, tag="hps")
                      for t in range(2):
                          for cp in range(3):
                              nc.tensor.matmul(hp[:, t, :], w1t[:, fp, t, cp],
                                               xT8[:, 2 * cp:2 * cp + 2]
                                               .rearrange("p c t f -> p c (t f)"),
                                               start=(cp == 0), stop=(cp == 2),
                                               perf_mode=PM.DoubleRow)
                      hsb = mhp.tile([128, 2, QTOK], FP8, tag="hsb")
                      if fp % 2 == 0:
                          nc.scalar.activation(hsb[:], hp[:], AF.Relu,
                                               scale=2.0 ** -6)
                      else:
                          nc.vector.tensor_scalar(hsb[:], hp[:], 2.0 ** -6,
                                                  0.0, op0=ALU.mult,
                                                  op1=ALU.max)
                      for tt in range(NTT):
                          for (cs, ce) in chunks:
                              nc.tensor.matmul(yps[tt][:, cs:ce],
                                               hsb[:, :, tt * 128:(tt + 1) * 128],
                                               w2t[:, fp, :, cs:ce],
                                               start=(fp == 0),
                                               stop=(zb and fp == FP - 1),
                                               perf_mode=PM.DoubleRow)
                  for tt in range(NTT):
                      if not zb:
                          for (cs, ce) in chunks:
                              nc.tensor.matmul(yps[tt][:, cs:ce], ones1b[0:1, :],
                                               b2r[0:1, e, cs:ce],
                                               start=False, stop=True)
                      if e == 0:
                          nc.scalar.mul(ffs[:, tt, :], yps[tt][:],
                                        comb[:, tt, e:e + 1])
                      else:
                          nc.vector.scalar_tensor_tensor(
                              ffs[:, tt, :], yps[tt][:], comb[:, tt, e:e + 1],
                              ffs[:, tt, :], op0=ALU.mult, op1=ALU.add)

              for tt in range(NTT):
                  pre2 = stp2.tile([128, D], F32, tag="pre")
                  nc.vector.tensor_tensor(pre2[:], x_sb[:, tt, :], ffs[:, tt, :],
                                          op=ALU.add)
                  layer_norm(out_sb[:, tt, :], pre2[:], 1)
              for tt in range(NTT):
                  nc.sync.dma_start(
                      d_out.ap()[tt * 128:(tt + 1) * 128].rearrange(
                          "(o p) d -> p o d", p=128), out_sb[:, tt:tt + 1, :])
        wpool.release()
        perpool.release()
        mwp0.release()
        bpool.release()
        cpool.release()

    nc.compile()
    return nc


def _prep(inputs):
    f = lambda a: np.ascontiguousarray(np.asarray(a, dtype=np.float32))
    bf = lambda a: np.ascontiguousarray(a).astype(ml_dtypes.bfloat16)
    src = f(inputs["src"])
    scale = (D // H) ** -0.5
    lnrows = np.concatenate([
        f(inputs["ln1_g"]), f(inputs["ln1_b"]),
        f(inputs["ln2_g"]), f(inputs["ln2_b"]),
        f(inputs["bv"]), f(inputs["bo"])]).reshape(1, 6 * D)
    common = {
        "wqT": bf(f(inputs["Wq"]).T * scale),
        "wkT": np.ascontiguousarray(f(inputs["Wk"]).T * 16.0).astype(
            np.dtype("float8_e4m3")),
        "wvT": np.ascontiguousarray(f(inputs["Wv"]).T * 16.0).astype(
            np.dtype("float8_e4m3")),
        "woT": bf(f(inputs["Wo"]).T),
        "bqc": (f(inputs["bq"]) * scale).reshape(DC, 128).T.copy(),
        "bkc": f(inputs["bk"]).reshape(DC, 128).T.copy(),
        "lnrows": bf(lnrows),
        "wgT": np.ascontiguousarray(f(inputs["Wg"]).T),
        "bgr": f(inputs["bg"]).reshape(1, E),
        "w1": np.ascontiguousarray(
            (f(inputs["W1"]) * 256.0).reshape(E, 3, 2, 128, FP, 2, 128)
            .transpose(0, 3, 4, 5, 1, 2, 6)).astype(np.dtype("float8_e4m3")),
        "w2": np.ascontiguousarray(
            (f(inputs["W2"]) * 256.0).reshape(E, FP, 2, 128, D)
            .transpose(0, 3, 1, 2, 4)).astype(np.dtype("float8_e4m3")),
        "b2r": bf(f(inputs["b2"]).reshape(1, E, D) * (2.0 ** 14)),
        "ident": np.eye(128, dtype=np.float32),
        "ones_r": np.ones((1, 128), dtype=np.float32),
    }
    assert not np.any(f(inputs["b1"])), "fp8 MoE path requires zero b1"
    in_maps = []
    for c in range(NCORES):
        b, qq = c // 4, c % 4
        m = dict(common)
        # rotate key/value token axis so this core's quarter sits at cols 0:256
        m["srcT"] = bf(np.roll(src[b].T, -qq * QTOK, axis=1))
        m["srcq"] = np.ascontiguousarray(src[b, qq * QTOK:(qq + 1) * QTOK])
        in_maps.append(m)
    return in_maps


def kernel(**inputs):
    global LAST_RESULT
    gamma = float(np.asarray(inputs["gamma"]))
    zb = (not any(np.any(np.asarray(inputs[k])) for k in
                  ("bq", "bk", "bv", "bo", "bg", "b2", "ln1_b", "ln2_b"))
          and all(np.all(np.asarray(inputs[k]) == 1.0) for k in ("ln1_g", "ln2_g")))
    key = (round(gamma, 9), zb)
    if key not in _cache:
        _cache[key] = _build(gamma, zb)
    nc = _cache[key]
    in_maps = _prep(inputs)
    trace = bool(os.environ.get("KERNEL_TRACE"))
    try:
        res = run_bass_kernel_spmd(nc, in_maps, list(range(NCORES)), trace=trace)
    except ModuleNotFoundError:
        res = run_bass_kernel_spmd(nc, in_maps, list(range(NCORES)), trace=False)
    LAST_RESULT = res
    out = np.empty((B, T, D), dtype=np.float32)
    for c in range(NCORES):
        b, qq = c // 4, c % 4
        out[b, qq * QTOK:(qq + 1) * QTOK] = res.results[c]["out"]
    return out
